# revision 26
# baseline (speedup 1.0000x reference)
"""DGCNN forward kernel for 8 Trainium2 NeuronCores.

Strategy: shard by graph (32 graphs/core). Message passing out[d] =
norm[d] * sum_s A[s,d] z[s] is computed as dense per-graph matmuls on the
TensorEngine with the adjacency streamed from HBM exactly once per graph:
all four conv layers are fused per block of 4 graphs, so A lives in SBUF
across layers. Layer-0's input projection z0 = x@W0.T + b0 is precomputed
on host (tiny GEMM); layers 1-3 compute z on-device via a block-diagonal
weight matmul covering 4 graphs at once. The per-graph accumulation
matmuls (M=32) for the 4 graphs of a block are packed into distinct PE
column groups (tile_position) so their N=512 streams run concurrently.

Precision: the host sort-pool sorts nodes by the h3 channel, so the key
needs fp32-class accuracy end-to-end (bf16 features reorder near-ties and
wreck the output -- measured 0.5 rel err). AMODE selects:
  zsplit - adjacency as exact integer counts in bf16 (half the HBM
           traffic of fp32) and z represented as a hi+lo bf16 pair, so
           acc = A^T z_hi + A^T z_lo reproduces fp32 to ~3e-6 (measured
           on HW) while streaming at bf16 rate. norm applied post-matmul
           on VectorE; h kept fp32 on-chip.
  f32    - everything fp32 with norm folded into A (baseline-class).
Sort-pool + tiny dense head run on host.
"""
import os
import sys
import numpy as np

if "/opt/trn_rl_repo" not in sys.path:
    sys.path.insert(0, "/opt/trn_rl_repo")

import ml_dtypes
import concourse.bass as bass
import concourse.mybir as mybir
from concourse.tile import TileContext
from concourse.vector_clock import ScopedClock, VectorClock
from concourse.bass_utils import run_bass_kernel_spmd

BF16NP = ml_dtypes.bfloat16
AMODE = os.environ.get("KERNEL_AMODE", "zsplit")

# ---------------- tile/walrus compatibility patches ----------------
_split_counter = [0]


def _drain_and_barrier(self, tick_clock, wait_clock):
    gc = tick_clock.global_clock
    n = len(gc)
    for i in range(n):
        if gc[i] > 0:
            vec = [0] * n
            vec[i] = gc[i]
            d = self.nc.sync.drain()
            wait_clock.add_sem_waits(d.ins, ScopedClock({None: VectorClock(vec)}))
    self.nc.all_engine_barrier()
    assert self.sems is not None
    popped = self.nc._tile_sem_poison_stack.pop()
    assert popped is self._sem_poison
    self.nc.clear_and_free_semaphores(list(self.sems.allocated().values()))
    self.nc.all_engine_barrier()


TileContext._drain_and_barrier = _drain_and_barrier


def _split_multi_waits(nc):
    """This walrus accepts at most one sync-wait per instruction; hoist
    extras onto InstNoOp instructions inserted before, same engine."""
    for f in nc.m.functions:
        for blk in f.blocks:
            insts = list(blk.instructions)
            if not any(
                i.sync_info is not None and len(i.sync_info.on_wait) > 1
                for i in insts
            ):
                continue
            new = []
            for inst in insts:
                si = inst.sync_info
                if si is not None and len(si.on_wait) > 1:
                    waits = list(si.on_wait)
                    for w in waits[:-1]:
                        _split_counter[0] += 1
                        nop = mybir.InstNoOp(
                            name=f"I-wsplit-{_split_counter[0]}", ins=[], outs=[]
                        )
                        nop.engine = inst.engine
                        nop.sync_info = mybir.SyncInfo(on_wait=[w], on_update=[])
                        new.append(nop)
                    inst.sync_info = mybir.SyncInfo(
                        on_wait=[waits[-1]], on_update=list(si.on_update)
                    )
                new.append(inst)
            blk.instructions = new


# ---------------- problem constants ----------------
B, NPER, DIMF, K = 256, 512, 128, 64
EMB = 32
NCORES = 8
GPC = B // NCORES          # graphs per core = 32
BLKG = 4                   # graphs per block (packed into 128 partitions)
NBLK = GPC // BLKG         # blocks per core = 8
FP32 = mybir.dt.float32
BF16 = mybir.dt.bfloat16
FP16 = mybir.dt.float16
FP8 = mybir.dt.float8e4          # e4m3: integers <= 16 are exact

_CACHE = {}


def _build_nc(with_bias, amode):
    key = ("nc", with_bias, amode)
    if key in _CACHE:
        return _CACHE[key]
    zs = amode == "zsplit"
    a_dt = FP8 if zs else FP32
    nc = bass.Bass("TRN2", target_bir_lowering=False, debug=False)
    # A tiles: block blk, graph j, src chunk c -> [128, 512] at col
    # (blk*16 + j*4 + c)*512. partition = src node within chunk.
    Ad = nc.dram_tensor("Ad", [128, GPC * 4 * NPER], a_dt, kind="ExternalInput")
    # z0: per block [128, 512] (f32) or [128, 1024] hi:lo (zsplit);
    # within a half, chunk c / graph j / emb e at col c*128 + j*32 + e.
    z0w = 1024 if zs else 512
    Z0d = nc.dram_tensor("Z0d", [128, NBLK * z0w], FP16 if zs else FP32,
                         kind="ExternalInput")
    # block-diagonal W_k^T for layers 1..3 (W3 zero-padded to 32 cols)
    Wd = nc.dram_tensor("Wd", [128, 3 * 128], FP32, kind="ExternalInput")
    if zs:
        # normrep: block blk -> [128, 512]; row 32j+e = norm of graph blk*4+j
        Nd = nc.dram_tensor("Nd", [128, NBLK * 512], FP32, kind="ExternalInput")
    if with_bias:
        Cd = nc.dram_tensor("Cd", [1, GPC * NPER], FP32, kind="ExternalInput")
        Bd = nc.dram_tensor("Bd", [1, 3 * 32], FP32, kind="ExternalInput")
    houts = [
        nc.dram_tensor(f"H{k}", [128, NBLK * NPER], FP16 if zs else BF16,
                       kind="ExternalOutput")
        for k in range(3)
    ]
    H3 = nc.dram_tensor("H3", [128, NBLK * NPER], FP32, kind="ExternalOutput")

    with TileContext(nc) as tc:
        with (
            tc.tile_pool(name="const", bufs=1) as constp,
            tc.tile_pool(name="ap", bufs=24) as apool,
            tc.tile_pool(name="z0p", bufs=6) as z0p,
            tc.tile_pool(name="np_", bufs=6) as nrp,
            tc.tile_pool(name="zsbp", bufs=6) as zsbp,
            tc.tile_pool(name="tp", bufs=6) as tmp_p,
            tc.tile_pool(name="hp", bufs=12) as hp,
            tc.tile_pool(name="pacc", bufs=4, space="PSUM") as pacc,
            tc.tile_pool(name="pz", bufs=3, space="PSUM") as pz,
        ):
            wsb = constp.tile([128, 3 * 128], FP32)
            nc.sync.dma_start(wsb[:], Wd[:])
            if with_bias:
                csb = constp.tile([1, GPC * NPER], FP32)
                nc.sync.dma_start(csb[:], Cd[:])
                bsb = constp.tile([1, 3 * 32], FP32)
                nc.sync.dma_start(bsb[:], Bd[:])

            def load_block(blk):
                at = []
                for j in range(4):
                    atj = apool.tile([128, 4 * NPER], a_dt, tag="at")
                    g16 = (blk * 4 + j) * 4
                    nc.sync.dma_start(
                        atj[:], Ad[:, g16 * NPER:(g16 + 4) * NPER])
                    at.append(atj)
                z0sb = z0p.tile([128, z0w], FP16 if zs else FP32, tag="z0")
                nc.sync.dma_start(z0sb[:], Z0d[:, blk * z0w:(blk + 1) * z0w])
                nrm = None
                if zs:
                    nrm = nrp.tile([128, 512], FP32, tag="nrm")
                    nc.sync.dma_start(nrm[:], Nd[:, blk * 512:(blk + 1) * 512])
                return {"at": at, "z0": z0sb, "nrm": nrm, "ht": None}

            def layer_z(st, blk, k):
                """z matmuls + hi/lo split for layer k>0 of a block."""
                zps = pz.tile([128, 512], FP32, tag="zps")
                for c in range(4):
                    nc.tensor.matmul(
                        zps[:, c * 128:(c + 1) * 128],
                        lhsT=st["ht"][:, c * 128:(c + 1) * 128],
                        rhs=wsb[:, (k - 1) * 128:k * 128],
                        start=True, stop=True)
                if zs:
                    zhi = zsbp.tile([128, 512], FP16, tag="zhi")
                    nc.scalar.activation(
                        zhi[:], zps[:], mybir.ActivationFunctionType.Copy)
                    zlo = zsbp.tile([128, 512], FP16, tag="zlo")
                    nc.vector.tensor_sub(zlo[:], zps[:], zhi[:])
                    return zhi, zlo
                zf = zsbp.tile([128, 512], FP32, tag="zf")
                nc.vector.tensor_copy(zf[:], zps[:])
                return zf, None

            def acc_mm_list(st, k, zhi, zlo, acc):
                """matmul arg list for one block's accumulation batch."""
                last_mm = not (with_bias and k > 0)
                mms = []
                for c in range(4):
                    for j in range(4):
                        sl = slice(c * 128 + 32 * j, c * 128 + 32 * j + 32)
                        rh = st["at"][j][:, c * NPER:(c + 1) * NPER]
                        mms.append((acc[32 * j:32 * j + 32, :], zhi[:, sl], rh,
                                    c == 0, c == 3 and not zs and last_mm,
                                    32 * j))
                if zs:
                    for c in range(4):
                        for j in range(4):
                            sl = slice(c * 128 + 32 * j, c * 128 + 32 * j + 32)
                            rh = st["at"][j][:, c * NPER:(c + 1) * NPER]
                            mms.append((acc[32 * j:32 * j + 32, :], zlo[:, sl],
                                        rh, False, c == 3 and last_mm, 32 * j))
                return mms

            def layer_post(st, blk, k, acc):
                """bias + norm + tanh + output DMA after the acc matmuls."""
                if with_bias and k > 0:
                    for j in range(4):
                        g = blk * 4 + j
                        nc.tensor.matmul(
                            acc[32 * j:32 * j + 32, :],
                            lhsT=bsb[0:1, (k - 1) * 32:k * 32],
                            rhs=csb[0:1, g * NPER:(g + 1) * NPER],
                            start=False, stop=True,
                            tile_position=(0, 32 * j))
                ht = hp.tile([128, 512], FP32, tag="ht")
                if zs:
                    tmp = tmp_p.tile([128, 512], FP32, tag="tmp")
                    nc.vector.tensor_mul(tmp[:], acc[:], st["nrm"][:])
                    nc.scalar.activation(
                        ht[:], tmp[:], mybir.ActivationFunctionType.Tanh)
                else:
                    nc.scalar.activation(
                        ht[:], acc[:], mybir.ActivationFunctionType.Tanh)
                if k < 3:
                    nc.gpsimd.dma_start(
                        houts[k][:, blk * NPER:(blk + 1) * NPER], ht[:])
                    st["ht"] = ht
                else:
                    nc.sync.dma_start(
                        H3[:, blk * NPER:(blk + 1) * NPER], ht[:])

            # Process blocks in pairs, layers staggered, so one block's PE
            # matmuls overlap the other's DVE/ACT post-processing. Groups
            # of 3 give the PE enough foreign work to fully hide each
            # block's mul+tanh chain (2 blocks leave a ~0.6us hole/layer).
            groups = [(0, 1, 2, 3), (4, 5, 6, 7)]
            for group in groups:
                states = [(b, load_block(b)) for b in group]
                for k in range(4):
                    # emit all blocks' z paths first so one block's hi/lo
                    # split (ACT+DVE) hides under the others' PE matmuls
                    zz = []
                    for blk, st in states:
                        if k == 0:
                            zz.append((st["z0"][:, 0:512],
                                       st["z0"][:, 512:1024] if zs else None))
                        else:
                            zz.append(layer_z(st, blk, k))
                    for (blk, st), (zhi, zlo) in zip(states, zz):
                        acc = pacc.tile([128, 512], FP32, tag="acc",
                                        name=f"acc{k}_{blk}")
                        for out, lh, rh, st_, sp_, tp in acc_mm_list(
                                st, k, zhi, zlo, acc):
                            nc.tensor.matmul(
                                out, lhsT=lh, rhs=rh, start=st_, stop=sp_,
                                tile_position=(0, tp))
                        layer_post(st, blk, k, acc)

    _split_multi_waits(nc)
    _CACHE[key] = nc
    return nc


def _host_prep(x, edge_src, edge_dst, Ws, bs, amode):
    zs = amode == "zsplit"
    src = np.asarray(edge_src).astype(np.int64).ravel()
    dst = np.asarray(edge_dst).astype(np.int64).ravel()
    N = B * NPER
    s_all = np.concatenate([src, np.arange(N)])
    d_all = np.concatenate([dst, np.arange(N)])
    deg = np.bincount(s_all, minlength=N).astype(np.float64)
    norm = (1.0 / deg).astype(np.float32)
    g = s_all // NPER
    flat = g * NPER * NPER + (s_all % NPER) * NPER + (d_all % NPER)
    A = np.bincount(flat, minlength=B * NPER * NPER).astype(np.float32)
    A = A.reshape(B, NPER, NPER)
    indeg = A.sum(axis=1).reshape(N)
    if zs:
        # e4m3 represents integers up to 16 exactly; edge multiplicities
        # here are tiny (Poisson ~0.03 + self-loop). Guarded at runtime.
        assert A.max() <= 16.0, "adjacency count exceeds fp8 exact range"
        A = A.astype(ml_dtypes.float8_e4m3)
        bias_vec = indeg.astype(np.float32)
    else:
        A *= norm.reshape(B, 1, NPER)      # fold norm[dst] into A columns
        bias_vec = (norm * indeg).astype(np.float32)

    # z0 = x @ W0.T + b0, node-major
    z0 = (np.asarray(x, np.float32) @ Ws[0].T.astype(np.float32)
          + bs[0][None, :]).astype(np.float32)         # [N, 32]

    # block-diagonal W_k^T (layers 1..3); W3^T zero-padded to 32 cols
    Wblk = np.zeros((128, 3 * 128), np.float32)
    for k in (1, 2, 3):
        wt = np.zeros((32, 32), np.float32)
        wkT = Ws[k].T                                  # [32, out]
        wt[:, :wkT.shape[1]] = wkT
        for j in range(4):
            Wblk[32 * j:32 * j + 32,
                 (k - 1) * 128 + 32 * j:(k - 1) * 128 + 32 * j + 32] = wt

    bcols = np.zeros((1, 3 * 32), np.float32)
    for k in (1, 2, 3):
        bk = np.zeros(32, np.float32)
        bk[: bs[k].shape[0]] = bs[k]
        bcols[0, (k - 1) * 32:k * 32] = bk
    return A, z0, Wblk, bcols, bias_vec, norm


def _run_mp(x, edge_src, edge_dst, Ws, bs):
    amode = AMODE
    zs = amode == "zsplit"
    A, z0, Wblk, bcols, bias_vec, norm = _host_prep(
        x, edge_src, edge_dst, Ws, bs, amode)
    with_bias = any(np.any(np.asarray(b) != 0) for b in bs[1:])
    nc = _build_nc(with_bias, amode)
    in_maps = []
    for c in range(NCORES):
        gs = slice(c * GPC, (c + 1) * GPC)
        ns = slice(c * GPC * NPER, (c + 1) * GPC * NPER)
        # A: [32,512,512] -> [blk, j, cchunk, p, d] -> [p, blk, j, cchunk, d]
        Ac = A[gs].reshape(NBLK, BLKG, 4, 128, NPER)
        Ad = np.ascontiguousarray(
            Ac.transpose(3, 0, 1, 2, 4)).reshape(128, -1)
        # z0: [16384, 32] -> [blk, j, cchunk, p, e] -> [p, blk, cchunk, j, e]
        z0c = z0[ns].reshape(NBLK, BLKG, 4, 128, EMB)
        if zs:
            z0t = z0c.transpose(3, 0, 2, 1, 4).reshape(128, NBLK, 512)
            z0hi = z0t.astype(np.float16)
            z0lo = (z0t - z0hi.astype(np.float32)).astype(np.float16)
            Z0c = np.ascontiguousarray(
                np.concatenate([z0hi, z0lo], axis=2)).reshape(128, -1)
        else:
            Z0c = np.ascontiguousarray(
                z0c.transpose(3, 0, 2, 1, 4)).reshape(128, -1)
        m = {"Ad": Ad, "Z0d": Z0c, "Wd": Wblk}
        if zs:
            nr = np.broadcast_to(
                norm[ns].reshape(NBLK, BLKG, 1, NPER), (NBLK, BLKG, 32, NPER))
            m["Nd"] = np.ascontiguousarray(
                nr.transpose(1, 2, 0, 3)).reshape(128, -1)
        if with_bias:
            m["Cd"] = bias_vec[ns].reshape(1, -1)
            m["Bd"] = bcols
        in_maps.append(m)
    trace = bool(int(os.environ.get("KERNEL_TRACE", "0")))
    if trace:
        _install_axon_hooks_shim()
    res = run_bass_kernel_spmd(
        nc, in_maps, core_ids=list(range(NCORES)), trace=trace)
    if trace and res.exec_time_ns is not None:
        print(f"HW exec time: {res.exec_time_ns} ns")
    hs = []
    for k in range(4):
        parts = []
        for c in range(NCORES):
            if k == 3:
                ht = np.asarray(res.results[c]["H3"])  # [128, NBLK*512]
                a = ht.reshape(BLKG, 32, NBLK, NPER)[:, 0].astype(np.float32)
                parts.append(a.transpose(1, 0, 2).reshape(-1, 1))
            else:
                ht = np.asarray(res.results[c][f"H{k}"])  # [128, NBLK*512]
                a = ht.reshape(BLKG, 32, NBLK, NPER).astype(np.float32)
                parts.append(a.transpose(2, 0, 3, 1).reshape(-1, EMB))
        hs.append(np.concatenate(parts, axis=0))
    return hs


def _install_axon_hooks_shim():
    import contextlib
    import ctypes
    import types
    if "antenv.axon_hooks" in sys.modules:
        return
    so = "/opt/axon/libaxon_pjrt.so"

    def make():
        lib = ctypes.CDLL(so)
        if not hasattr(lib, "axon_start_nrt_profile"):
            return None
        lib.axon_start_nrt_profile.argtypes = [
            ctypes.POINTER(ctypes.c_int64), ctypes.c_size_t]
        lib.axon_start_nrt_profile.restype = ctypes.c_int64
        lib.axon_stop_nrt_profile.argtypes = [ctypes.c_char_p]
        lib.axon_stop_nrt_profile.restype = ctypes.c_int64

        @contextlib.contextmanager
        def hook(output_dir, device_ids):
            import jax
            jax.devices()
            if device_ids:
                ids = (ctypes.c_int64 * len(device_ids))(*device_ids)
                rc = lib.axon_start_nrt_profile(ids, len(device_ids))
            else:
                rc = lib.axon_start_nrt_profile(None, 0)
            if rc != 0:
                raise RuntimeError(f"start profile rc={rc}")
            try:
                yield
            finally:
                lib.axon_stop_nrt_profile(str(output_dir).encode())

        return hook

    mod = types.ModuleType("antenv.axon_hooks")
    h = make()
    mod.get_axon_ntff_profile_hook = lambda: h
    mod.set_axon_ntff_profile_hook = lambda hh: None
    sys.modules["antenv.axon_hooks"] = mod


def kernel(**inputs):
    x = np.asarray(inputs["x"], np.float32)
    Ws = [np.asarray(inputs[f"W{i}"], np.float32) for i in range(4)]
    bs = [np.asarray(inputs[f"b{i}"], np.float32) for i in range(4)]
    hs = _run_mp(x, inputs["edge_src"], inputs["edge_dst"], Ws, bs)
    # ---- sort-pool + head (small, host) ----
    feat = np.concatenate([hs[0], hs[1], hs[2], hs[3][:, :1]], axis=1)  # [N, 97]
    key = hs[3][:, 0].reshape(B, NPER)
    order = np.argsort(-key, axis=1, kind="stable")[:, :K]
    topk = np.take_along_axis(feat.reshape(B, NPER, 97), order[:, :, None], axis=1)
    w1 = np.asarray(inputs["conv1_w"], np.float32)[:, 0, :]
    c1 = np.einsum("bkd,od->bok", topk, w1) + np.asarray(inputs["conv1_b"], np.float32)[None, :, None]
    c1 = np.maximum(c1, 0)
    p = c1.reshape(B, 16, K // 2, 2).max(axis=-1)
    w2 = np.asarray(inputs["conv2_w"], np.float32)
    c2 = np.zeros((B, 32, 28), np.float32)
    for t in range(28):
        c2[:, :, t] = np.einsum("bis,ois->bo", p[:, :, t:t + 5], w2)
    c2 = np.maximum(c2 + np.asarray(inputs["conv2_b"], np.float32)[None, :, None], 0)
    flat = c2.reshape(B, -1)
    hid = np.maximum(flat @ np.asarray(inputs["d1_w"], np.float32).T
                     + np.asarray(inputs["d1_b"], np.float32), 0)
    out = hid @ np.asarray(inputs["d2_w"], np.float32).T + np.asarray(inputs["d2_b"], np.float32)
    return out.astype(np.float32)


# revision 30
# speedup vs baseline: 1.0844x; 1.0844x over previous
"""DGCNN forward kernel for 8 Trainium2 NeuronCores.

Strategy: shard by graph (32 graphs/core). Message passing out[d] =
norm[d] * sum_s A[s,d] z[s] is computed as dense per-graph matmuls on the
TensorEngine with the adjacency streamed from HBM exactly once per graph:
all four conv layers are fused per block of 4 graphs, so A lives in SBUF
across layers. Layer-0's input projection z0 = x@W0.T + b0 is precomputed
on host (tiny GEMM); layers 1-3 compute z on-device via a block-diagonal
weight matmul covering 4 graphs at once. The per-graph accumulation
matmuls (M=32) for the 4 graphs of a block are packed into distinct PE
column groups (tile_position) so their N=512 streams run concurrently.

Precision: the host sort-pool sorts nodes by the h3 channel, so the key
needs fp32-class accuracy end-to-end (bf16 features reorder near-ties and
wreck the output -- measured 0.5 rel err). AMODE selects:
  zsplit - adjacency as exact integer counts in bf16 (half the HBM
           traffic of fp32) and z represented as a hi+lo bf16 pair, so
           acc = A^T z_hi + A^T z_lo reproduces fp32 to ~3e-6 (measured
           on HW) while streaming at bf16 rate. norm applied post-matmul
           on VectorE; h kept fp32 on-chip.
  f32    - everything fp32 with norm folded into A (baseline-class).
Sort-pool + tiny dense head run on host.
"""
import os
import sys
import numpy as np

if "/opt/trn_rl_repo" not in sys.path:
    sys.path.insert(0, "/opt/trn_rl_repo")

import ml_dtypes
import concourse.bass as bass
import concourse.mybir as mybir
from concourse.tile import TileContext
from concourse.vector_clock import ScopedClock, VectorClock
from concourse.bass_utils import run_bass_kernel_spmd

BF16NP = ml_dtypes.bfloat16
AMODE = os.environ.get("KERNEL_AMODE", "zsplit")

# ---------------- tile/walrus compatibility patches ----------------
_split_counter = [0]


def _drain_and_barrier(self, tick_clock, wait_clock):
    gc = tick_clock.global_clock
    n = len(gc)
    for i in range(n):
        if gc[i] > 0:
            vec = [0] * n
            vec[i] = gc[i]
            d = self.nc.sync.drain()
            wait_clock.add_sem_waits(d.ins, ScopedClock({None: VectorClock(vec)}))
    self.nc.all_engine_barrier()
    assert self.sems is not None
    popped = self.nc._tile_sem_poison_stack.pop()
    assert popped is self._sem_poison
    self.nc.clear_and_free_semaphores(list(self.sems.allocated().values()))
    self.nc.all_engine_barrier()


TileContext._drain_and_barrier = _drain_and_barrier


def _split_multi_waits(nc):
    """This walrus accepts at most one sync-wait per instruction; hoist
    extras onto InstNoOp instructions inserted before, same engine."""
    for f in nc.m.functions:
        for blk in f.blocks:
            insts = list(blk.instructions)
            if not any(
                i.sync_info is not None and len(i.sync_info.on_wait) > 1
                for i in insts
            ):
                continue
            new = []
            for inst in insts:
                si = inst.sync_info
                if si is not None and len(si.on_wait) > 1:
                    waits = list(si.on_wait)
                    for w in waits[:-1]:
                        _split_counter[0] += 1
                        nop = mybir.InstNoOp(
                            name=f"I-wsplit-{_split_counter[0]}", ins=[], outs=[]
                        )
                        nop.engine = inst.engine
                        nop.sync_info = mybir.SyncInfo(on_wait=[w], on_update=[])
                        new.append(nop)
                    inst.sync_info = mybir.SyncInfo(
                        on_wait=[waits[-1]], on_update=list(si.on_update)
                    )
                new.append(inst)
            blk.instructions = new


# ---------------- problem constants ----------------
B, NPER, DIMF, K = 256, 512, 128, 64
EMB = 32
NCORES = 8
GPC = B // NCORES          # graphs per core = 32
BLKG = 4                   # graphs per block (packed into 128 partitions)
NBLK = GPC // BLKG         # blocks per core = 8
FP32 = mybir.dt.float32
BF16 = mybir.dt.bfloat16
FP16 = mybir.dt.float16
FP8 = mybir.dt.float8e4          # e4m3: integers <= 16 are exact

_CACHE = {}


def _build_nc(with_bias, amode):
    key = ("nc", with_bias, amode)
    if key in _CACHE:
        return _CACHE[key]
    zs = amode == "zsplit"
    a_dt = FP8 if zs else FP32
    nc = bass.Bass("TRN2", target_bir_lowering=False, debug=False)
    # A tiles: block blk, graph j, src chunk c -> [128, 512] at col
    # (blk*16 + j*4 + c)*512. partition = src node within chunk.
    Ad = nc.dram_tensor("Ad", [128, GPC * 4 * NPER], a_dt, kind="ExternalInput")
    # z0: per block [128, 512] (f32) or [128, 1024] hi:lo (zsplit);
    # within a half, chunk c / graph j / emb e at col c*128 + j*32 + e.
    z0w = 1024 if zs else 512
    Z0d = nc.dram_tensor("Z0d", [128, NBLK * z0w], FP16 if zs else FP32,
                         kind="ExternalInput")
    # block-diagonal W_k^T for layers 1..3 (W3 zero-padded to 32 cols)
    Wd = nc.dram_tensor("Wd", [128, 3 * 128], FP32, kind="ExternalInput")
    if zs:
        # normrep: block blk -> [128, 512]; row 32j+e = norm of graph blk*4+j
        Nd = nc.dram_tensor("Nd", [128, NBLK * 512], FP32, kind="ExternalInput")
    if with_bias:
        Cd = nc.dram_tensor("Cd", [1, GPC * NPER], FP32, kind="ExternalInput")
        Bd = nc.dram_tensor("Bd", [1, 3 * 32], FP32, kind="ExternalInput")
    houts = [
        nc.dram_tensor(f"H{k}", [128, NBLK * NPER], FP16 if zs else BF16,
                       kind="ExternalOutput")
        for k in range(3)
    ]
    H3 = nc.dram_tensor("H3", [128, NBLK * NPER], FP32, kind="ExternalOutput")

    with TileContext(nc) as tc:
        with (
            tc.tile_pool(name="const", bufs=1) as constp,
            tc.tile_pool(name="ap", bufs=24) as apool,
            tc.tile_pool(name="z0p", bufs=6) as z0p,
            tc.tile_pool(name="np_", bufs=6) as nrp,
            tc.tile_pool(name="zsbp", bufs=6) as zsbp,
            tc.tile_pool(name="tp", bufs=6) as tmp_p,
            tc.tile_pool(name="hp", bufs=12) as hp,
            tc.tile_pool(name="pacc", bufs=4, space="PSUM") as pacc,
            tc.tile_pool(name="pz", bufs=3, space="PSUM") as pz,
        ):
            wsb = constp.tile([128, 3 * 128], FP32)
            nc.sync.dma_start(wsb[:], Wd[:])
            if with_bias:
                csb = constp.tile([1, GPC * NPER], FP32)
                nc.sync.dma_start(csb[:], Cd[:])
                bsb = constp.tile([1, 3 * 32], FP32)
                nc.sync.dma_start(bsb[:], Bd[:])

            def load_block(blk):
                at = []
                for j in range(4):
                    atj = apool.tile([128, 4 * NPER], a_dt, tag="at")
                    g16 = (blk * 4 + j) * 4
                    nc.sync.dma_start(
                        atj[:], Ad[:, g16 * NPER:(g16 + 4) * NPER])
                    at.append(atj)
                z0sb = z0p.tile([128, z0w], FP16 if zs else FP32, tag="z0")
                nc.sync.dma_start(z0sb[:], Z0d[:, blk * z0w:(blk + 1) * z0w])
                nrm = None
                if zs:
                    nrm = nrp.tile([128, 512], FP32, tag="nrm")
                    nc.sync.dma_start(nrm[:], Nd[:, blk * 512:(blk + 1) * 512])
                return {"at": at, "z0": z0sb, "nrm": nrm, "ht": None}

            def layer_z(st, blk, k):
                """z matmuls + hi/lo split for layer k>0 of a block.
                Layer 3 has a single real output channel per graph, so its
                z is only [128, 4] (graph j at col 4c+j per chunk)."""
                w = 512 if k < 3 else 16
                nsub = 128 if k < 3 else 4
                zps = pz.tile([128, w], FP32, tag="zps", name=f"zps{k}_{blk}")
                for c in range(4):
                    nc.tensor.matmul(
                        zps[:, c * nsub:(c + 1) * nsub],
                        lhsT=st["ht"][:, c * 128:(c + 1) * 128],
                        rhs=wsb[:, (k - 1) * 128:(k - 1) * 128 + nsub],
                        start=True, stop=True)
                if zs:
                    zhi = zsbp.tile([128, w], FP16, tag="zhi",
                                    name=f"zhi{k}_{blk}")
                    nc.scalar.activation(
                        zhi[:], zps[:], mybir.ActivationFunctionType.Copy)
                    zlo = zsbp.tile([128, w], FP16, tag="zlo",
                                    name=f"zlo{k}_{blk}")
                    nc.vector.tensor_sub(zlo[:], zps[:], zhi[:])
                    return zhi, zlo
                zf = zsbp.tile([128, w], FP32, tag="zf", name=f"zf{k}_{blk}")
                nc.vector.tensor_copy(zf[:], zps[:])
                return zf, None

            def acc_mm_list(st, k, zhi, zlo, acc):
                """matmul arg list for one block's accumulation batch."""
                last_mm = not (with_bias and k > 0)
                mw = 32 if k < 3 else 1     # layer-3 z is [128, 4]

                def zsl(c, j):
                    if k < 3:
                        return slice(c * 128 + 32 * j, c * 128 + 32 * j + 32)
                    return slice(c * 4 + j, c * 4 + j + 1)

                mms = []
                for c in range(4):
                    for j in range(4):
                        rh = st["at"][j][:, c * NPER:(c + 1) * NPER]
                        mms.append((acc[32 * j:32 * j + mw, :],
                                    zhi[:, zsl(c, j)], rh,
                                    c == 0, c == 3 and not zs and last_mm,
                                    32 * j))
                if zs:
                    for c in range(4):
                        for j in range(4):
                            rh = st["at"][j][:, c * NPER:(c + 1) * NPER]
                            mms.append((acc[32 * j:32 * j + mw, :],
                                        zlo[:, zsl(c, j)],
                                        rh, False, c == 3 and last_mm, 32 * j))
                return mms

            def layer_post(st, blk, k, acc):
                """bias + norm + tanh + output DMA after the acc matmuls."""
                if with_bias and k > 0:
                    for j in range(4):
                        g = blk * 4 + j
                        nc.tensor.matmul(
                            acc[32 * j:32 * j + 32, :],
                            lhsT=bsb[0:1, (k - 1) * 32:k * 32],
                            rhs=csb[0:1, g * NPER:(g + 1) * NPER],
                            start=False, stop=True,
                            tile_position=(0, 32 * j))
                ht = hp.tile([128, 512], FP32, tag="ht")
                if zs:
                    tmp = tmp_p.tile([128, 512], FP32, tag="tmp")
                    nc.vector.tensor_mul(tmp[:], acc[:], st["nrm"][:])
                    nc.scalar.activation(
                        ht[:], tmp[:], mybir.ActivationFunctionType.Tanh)
                else:
                    nc.scalar.activation(
                        ht[:], acc[:], mybir.ActivationFunctionType.Tanh)
                if k < 3:
                    nc.gpsimd.dma_start(
                        houts[k][:, blk * NPER:(blk + 1) * NPER], ht[:])
                    st["ht"] = ht
                else:
                    nc.sync.dma_start(
                        H3[:, blk * NPER:(blk + 1) * NPER], ht[:])

            # Process blocks in pairs, layers staggered, so one block's PE
            # matmuls overlap the other's DVE/ACT post-processing. Groups
            # of 3 give the PE enough foreign work to fully hide each
            # block's mul+tanh chain (2 blocks leave a ~0.6us hole/layer).
            groups = [(0, 1, 2), (3, 4, 5), (6, 7)]
            for group in groups:
                states = [(b, load_block(b)) for b in group]
                for k in range(4):
                    # emit all blocks' z paths first so one block's hi/lo
                    # split (ACT+DVE) hides under the others' PE matmuls
                    zz = []
                    for blk, st in states:
                        if k == 0:
                            zz.append((st["z0"][:, 0:512],
                                       st["z0"][:, 512:1024] if zs else None))
                        else:
                            zz.append(layer_z(st, blk, k))
                    for (blk, st), (zhi, zlo) in zip(states, zz):
                        acc = pacc.tile([128, 512], FP32, tag="acc",
                                        name=f"acc{k}_{blk}")
                        for out, lh, rh, st_, sp_, tp in acc_mm_list(
                                st, k, zhi, zlo, acc):
                            nc.tensor.matmul(
                                out, lhsT=lh, rhs=rh, start=st_, stop=sp_,
                                tile_position=(0, tp))
                        layer_post(st, blk, k, acc)

    _split_multi_waits(nc)
    _CACHE[key] = nc
    return nc


def _host_prep(x, edge_src, edge_dst, Ws, bs, amode):
    zs = amode == "zsplit"
    src = np.asarray(edge_src).astype(np.int64).ravel()
    dst = np.asarray(edge_dst).astype(np.int64).ravel()
    N = B * NPER
    s_all = np.concatenate([src, np.arange(N)])
    d_all = np.concatenate([dst, np.arange(N)])
    deg = np.bincount(s_all, minlength=N).astype(np.float64)
    norm = (1.0 / deg).astype(np.float32)
    g = s_all // NPER
    flat = g * NPER * NPER + (s_all % NPER) * NPER + (d_all % NPER)
    A = np.bincount(flat, minlength=B * NPER * NPER).astype(np.float32)
    A = A.reshape(B, NPER, NPER)
    indeg = A.sum(axis=1).reshape(N)
    if zs:
        # e4m3 represents integers up to 16 exactly; edge multiplicities
        # here are tiny (Poisson ~0.03 + self-loop). Guarded at runtime.
        assert A.max() <= 16.0, "adjacency count exceeds fp8 exact range"
        A = A.astype(ml_dtypes.float8_e4m3)
        bias_vec = indeg.astype(np.float32)
    else:
        A *= norm.reshape(B, 1, NPER)      # fold norm[dst] into A columns
        bias_vec = (norm * indeg).astype(np.float32)

    # z0 = x @ W0.T + b0, node-major
    z0 = (np.asarray(x, np.float32) @ Ws[0].T.astype(np.float32)
          + bs[0][None, :]).astype(np.float32)         # [N, 32]

    # block-diagonal W_k^T (layers 1..2); layer 3 is 1 output channel per
    # graph: col 256+j holds W3^T in rows 32j:32j+32.
    Wblk = np.zeros((128, 3 * 128), np.float32)
    for k in (1, 2):
        for j in range(4):
            Wblk[32 * j:32 * j + 32,
                 (k - 1) * 128 + 32 * j:(k - 1) * 128 + 32 * j + 32] = Ws[k].T
    for j in range(4):
        Wblk[32 * j:32 * j + 32, 256 + j] = Ws[3].T[:, 0]

    bcols = np.zeros((1, 3 * 32), np.float32)
    for k in (1, 2, 3):
        bk = np.zeros(32, np.float32)
        bk[: bs[k].shape[0]] = bs[k]
        bcols[0, (k - 1) * 32:k * 32] = bk
    return A, z0, Wblk, bcols, bias_vec, norm


def _run_mp(x, edge_src, edge_dst, Ws, bs):
    amode = AMODE
    zs = amode == "zsplit"
    A, z0, Wblk, bcols, bias_vec, norm = _host_prep(
        x, edge_src, edge_dst, Ws, bs, amode)
    with_bias = any(np.any(np.asarray(b) != 0) for b in bs[1:])
    nc = _build_nc(with_bias, amode)
    in_maps = []
    for c in range(NCORES):
        gs = slice(c * GPC, (c + 1) * GPC)
        ns = slice(c * GPC * NPER, (c + 1) * GPC * NPER)
        # A: [32,512,512] -> [blk, j, cchunk, p, d] -> [p, blk, j, cchunk, d]
        Ac = A[gs].reshape(NBLK, BLKG, 4, 128, NPER)
        Ad = np.ascontiguousarray(
            Ac.transpose(3, 0, 1, 2, 4)).reshape(128, -1)
        # z0: [16384, 32] -> [blk, j, cchunk, p, e] -> [p, blk, cchunk, j, e]
        z0c = z0[ns].reshape(NBLK, BLKG, 4, 128, EMB)
        if zs:
            z0t = z0c.transpose(3, 0, 2, 1, 4).reshape(128, NBLK, 512)
            z0hi = z0t.astype(np.float16)
            z0lo = (z0t - z0hi.astype(np.float32)).astype(np.float16)
            Z0c = np.ascontiguousarray(
                np.concatenate([z0hi, z0lo], axis=2)).reshape(128, -1)
        else:
            Z0c = np.ascontiguousarray(
                z0c.transpose(3, 0, 2, 1, 4)).reshape(128, -1)
        m = {"Ad": Ad, "Z0d": Z0c, "Wd": Wblk}
        if zs:
            nr = np.broadcast_to(
                norm[ns].reshape(NBLK, BLKG, 1, NPER), (NBLK, BLKG, 32, NPER))
            m["Nd"] = np.ascontiguousarray(
                nr.transpose(1, 2, 0, 3)).reshape(128, -1)
        if with_bias:
            m["Cd"] = bias_vec[ns].reshape(1, -1)
            m["Bd"] = bcols
        in_maps.append(m)
    trace = bool(int(os.environ.get("KERNEL_TRACE", "0")))
    if trace:
        _install_axon_hooks_shim()
    res = run_bass_kernel_spmd(
        nc, in_maps, core_ids=list(range(NCORES)), trace=trace)
    if trace and res.exec_time_ns is not None:
        print(f"HW exec time: {res.exec_time_ns} ns")
    hs = []
    for k in range(4):
        parts = []
        for c in range(NCORES):
            if k == 3:
                ht = np.asarray(res.results[c]["H3"])  # [128, NBLK*512]
                a = ht.reshape(BLKG, 32, NBLK, NPER)[:, 0].astype(np.float32)
                parts.append(a.transpose(1, 0, 2).reshape(-1, 1))
            else:
                ht = np.asarray(res.results[c][f"H{k}"])  # [128, NBLK*512]
                a = ht.reshape(BLKG, 32, NBLK, NPER).astype(np.float32)
                parts.append(a.transpose(2, 0, 3, 1).reshape(-1, EMB))
        hs.append(np.concatenate(parts, axis=0))
    return hs


def _install_axon_hooks_shim():
    import contextlib
    import ctypes
    import types
    if "antenv.axon_hooks" in sys.modules:
        return
    so = "/opt/axon/libaxon_pjrt.so"

    def make():
        lib = ctypes.CDLL(so)
        if not hasattr(lib, "axon_start_nrt_profile"):
            return None
        lib.axon_start_nrt_profile.argtypes = [
            ctypes.POINTER(ctypes.c_int64), ctypes.c_size_t]
        lib.axon_start_nrt_profile.restype = ctypes.c_int64
        lib.axon_stop_nrt_profile.argtypes = [ctypes.c_char_p]
        lib.axon_stop_nrt_profile.restype = ctypes.c_int64

        @contextlib.contextmanager
        def hook(output_dir, device_ids):
            import jax
            jax.devices()
            if device_ids:
                ids = (ctypes.c_int64 * len(device_ids))(*device_ids)
                rc = lib.axon_start_nrt_profile(ids, len(device_ids))
            else:
                rc = lib.axon_start_nrt_profile(None, 0)
            if rc != 0:
                raise RuntimeError(f"start profile rc={rc}")
            try:
                yield
            finally:
                lib.axon_stop_nrt_profile(str(output_dir).encode())

        return hook

    mod = types.ModuleType("antenv.axon_hooks")
    h = make()
    mod.get_axon_ntff_profile_hook = lambda: h
    mod.set_axon_ntff_profile_hook = lambda hh: None
    sys.modules["antenv.axon_hooks"] = mod


def kernel(**inputs):
    x = np.asarray(inputs["x"], np.float32)
    Ws = [np.asarray(inputs[f"W{i}"], np.float32) for i in range(4)]
    bs = [np.asarray(inputs[f"b{i}"], np.float32) for i in range(4)]
    hs = _run_mp(x, inputs["edge_src"], inputs["edge_dst"], Ws, bs)
    # ---- sort-pool + head (small, host) ----
    feat = np.concatenate([hs[0], hs[1], hs[2], hs[3][:, :1]], axis=1)  # [N, 97]
    key = hs[3][:, 0].reshape(B, NPER)
    order = np.argsort(-key, axis=1, kind="stable")[:, :K]
    topk = np.take_along_axis(feat.reshape(B, NPER, 97), order[:, :, None], axis=1)
    w1 = np.asarray(inputs["conv1_w"], np.float32)[:, 0, :]
    c1 = np.einsum("bkd,od->bok", topk, w1) + np.asarray(inputs["conv1_b"], np.float32)[None, :, None]
    c1 = np.maximum(c1, 0)
    p = c1.reshape(B, 16, K // 2, 2).max(axis=-1)
    w2 = np.asarray(inputs["conv2_w"], np.float32)
    c2 = np.zeros((B, 32, 28), np.float32)
    for t in range(28):
        c2[:, :, t] = np.einsum("bis,ois->bo", p[:, :, t:t + 5], w2)
    c2 = np.maximum(c2 + np.asarray(inputs["conv2_b"], np.float32)[None, :, None], 0)
    flat = c2.reshape(B, -1)
    hid = np.maximum(flat @ np.asarray(inputs["d1_w"], np.float32).T
                     + np.asarray(inputs["d1_b"], np.float32), 0)
    out = hid @ np.asarray(inputs["d2_w"], np.float32).T + np.asarray(inputs["d2_b"], np.float32)
    return out.astype(np.float32)
